# revision 1
# baseline (speedup 1.0000x reference)
"""Trainium2 Bass kernel for nn_DepthWiseConv_AConnect (depthwise 3x3 conv with
per-pool multiplicative weight/bias noise, followed by 8-bit LQuant).

Strategy (8 NeuronCores, data-parallel over the pool axis):
  - Core p handles pool group p: 8 images, Werr[p], Berr[p].
  - The workload is memory-bound, so inputs ship as fp16 (products of two
    11-bit-mantissa halves are exact in the fp32 PSUM accumulator) and the
    quantized outputs ship as int8 (the LQuant integer grid is int8-exact):
    19.4 MB/core moved instead of 49.6 MB.
  - Host pre-transposes X to channels-major [n, c, h*w] fp16.
  - On device, the depthwise conv is 9 accumulating diagonal matmuls on the
    TensorEngine per PSUM bank tile of 486 output pixels (9 output rows):
    psum[c, pix] += diag(127*w_t)[c] * X^T[c, pix + shift_t].
  - ScalarE evacuates PSUM adding the (pre-scaled) bias; VectorE adds the
    1.5*2^23 magic constant (forcing round-to-nearest-even onto the integer
    grid, matching jnp.round's half-to-even) fused with the upper clip;
    GPSIMD applies the lower clip, subtracts the magic back out, and emits
    int8.
  - Host divides by 127 and transposes back to NHWC.
"""
import sys

import numpy as np

try:
    import concourse.bacc as bacc_mod
except ImportError:
    sys.path.insert(0, "/opt/trn_rl_repo")
    import concourse.bacc as bacc_mod

import concourse.mybir as mybir
from concourse.tile import TileContext
from concourse.bass_utils import run_bass_kernel_spmd
from contextlib import ExitStack

POOL = 8
NB = 8            # images per pool group (64 / 8)
H = W = 56
HO = WO = 54
C = 256
NCH = 2           # channel chunks of 128
NPIX = H * W      # 3136
NOUT = HO * WO    # 2916
NBANK = 6         # psum bank tiles per plane (6 * 486 = 2916)
BANKN = 486       # output pixels per psum tile (9 rows x 54)
MAGIC = 12582912.0  # 1.5 * 2^23
S = 127.0

f32 = mybir.dt.float32
f16 = mybir.dt.float16
i8 = mybir.dt.int8
Alu = mybir.AluOpType
Act = mybir.ActivationFunctionType

_cached = {}


def _build():
    nc = bacc_mod.Bacc()
    xt = nc.dram_tensor("xt", [NB, NCH, 128, NPIX], f16, kind="ExternalInput")
    wdg = nc.dram_tensor("wdg", [NCH, 9, 128, 128], f16, kind="ExternalInput")
    bv = nc.dram_tensor("bv", [NCH, 128, 1], f32, kind="ExternalInput")
    out = nc.dram_tensor("out", [NB, NCH, 128, NOUT], i8, kind="ExternalOutput")

    with TileContext(nc) as tc, ExitStack() as ctx:
        consts = ctx.enter_context(tc.tile_pool(name="consts", bufs=1))
        xpool = ctx.enter_context(tc.tile_pool(name="xpool", bufs=3))
        ypool = ctx.enter_context(tc.tile_pool(name="ypool", bufs=2))
        tpool = ctx.enter_context(tc.tile_pool(name="tpool", bufs=2))
        opool = ctx.enter_context(tc.tile_pool(name="opool", bufs=2))
        pspool = ctx.enter_context(tc.tile_pool(name="pspool", bufs=8, space="PSUM"))

        ws = consts.tile([128, NCH, 9, 128], f16)
        nc.sync.dma_start(out=ws, in_=wdg.rearrange("q t k m -> k q t m"))
        bt = consts.tile([128, NCH, 1], f32)
        nc.sync.dma_start(out=bt, in_=bv.rearrange("q k o -> k q o"))

        for n in range(NB):
            for q in range(NCH):
                xs = xpool.tile([128, NPIX], f16, tag="xs")
                nc.sync.dma_start(out=xs, in_=xt[n, q])
                xr = xs.rearrange("p (h w) -> p h w", w=W)
                y = ypool.tile([128, NOUT], f32, tag="y")
                for b in range(NBANK):
                    ps = pspool.tile([128, BANKN], f32, tag="ps")
                    for t in range(9):
                        i, j = divmod(t, 3)
                        rhs = xr[:, 9 * b + i: 9 * b + i + 9, j: j + 54]
                        nc.tensor.matmul(ps, lhsT=ws[:, q, t, :], rhs=rhs,
                                         start=(t == 0), stop=(t == 8),
                                         skip_group_check=True)
                    nc.scalar.activation(out=y[:, BANKN * b: BANKN * (b + 1)],
                                         in_=ps, func=Act.Identity,
                                         bias=bt[:, q, :], scale=1.0)
                t1 = tpool.tile([128, NOUT], f32, tag="t1")
                nc.vector.tensor_scalar(out=t1, in0=y, scalar1=MAGIC,
                                        scalar2=MAGIC + S, op0=Alu.add,
                                        op1=Alu.min)
                ot = opool.tile([128, NOUT], i8, tag="ot")
                nc.gpsimd.tensor_scalar(out=ot, in0=t1, scalar1=MAGIC - S,
                                        scalar2=MAGIC, op0=Alu.max,
                                        op1=Alu.subtract)
                nc.sync.dma_start(out=out[n, q], in_=ot)

    nc.finalize()
    return nc


def kernel(X, W, bias, Werr, Berr, _trace=False):
    X = np.asarray(X, np.float32)
    W = np.asarray(W, np.float32)
    bias = np.asarray(bias, np.float32)
    Werr = np.asarray(Werr, np.float32)
    Berr = np.asarray(Berr, np.float32)

    if "nc" not in _cached:
        _cached["nc"] = _build()
    nc = _cached["nc"]

    Xh = X.astype(np.float16)  # [64, 56, 56, 256]
    w3 = W[..., 0]             # [3, 3, 256]
    we3 = Werr[..., 0]         # [8, 3, 3, 256]

    in_maps = []
    for p in range(POOL):
        xp = Xh[p * NB:(p + 1) * NB].reshape(NB, NPIX, C)
        xp = np.ascontiguousarray(xp.transpose(0, 2, 1)).reshape(NB, NCH, 128, NPIX)

        w_eff = (np.float32(S) * w3 * we3[p]).astype(np.float16)  # [3, 3, 256]
        wdg = np.zeros((NCH, 9, 128, 128), np.float16)
        for q in range(NCH):
            for t in range(9):
                i, j = divmod(t, 3)
                np.fill_diagonal(wdg[q, t], w_eff[i, j, 128 * q:128 * (q + 1)])

        b_eff = (np.float32(S) * bias * Berr[p]).astype(np.float32)
        in_maps.append({"xt": xp, "wdg": wdg, "bv": b_eff.reshape(NCH, 128, 1)})

    res = run_bass_kernel_spmd(nc, in_maps, core_ids=list(range(POOL)),
                               trace=_trace)
    if _trace:
        _cached["last_result"] = res

    outs = []
    for p in range(POOL):
        o = res.results[p]["out"].astype(np.float32)  # [NB, NCH, 128, NOUT] int8
        o = o / np.float32(S)
        o = o.reshape(NB, C, HO, WO).transpose(0, 2, 3, 1)  # NHWC
        outs.append(o)
    return np.ascontiguousarray(np.concatenate(outs, axis=0).astype(np.float32))



# revision 6
# speedup vs baseline: 3.4844x; 3.4844x over previous
"""Trainium2 Bass kernel for nn_DepthWiseConv_AConnect (depthwise 3x3 conv with
per-pool multiplicative weight/bias noise, followed by 8-bit LQuant).

Strategy (8 NeuronCores, data-parallel over the pool axis):
  - Core p handles pool group p: 8 images, Werr[p], Berr[p].
  - Inputs ship as fp16 (products of two 11-bit-mantissa halves are exact in
    the fp32 PSUM accumulator); quantized outputs ship as int8.
  - Host pre-transposes X to channels-major [n, c, h*w] fp16.
  - On device, the depthwise conv is 9 accumulating diagonal matmuls on the
    TensorEngine per PSUM bank tile of 486 output pixels (9 output rows):
    psum[c, pix] += diag(127*w_t)[c] * X^T[c, pix + shift_t].
    Matmuls run tap-outer over 3-bank halves so consecutive instructions
    share the stationary weights.
  - ScalarE evacuates PSUM per bank adding the (pre-scaled) bias at full
    fp32 precision; VectorE then quantizes per chunk in two fused
    tensor_scalar passes:
      ts1: (y + MAGIC) min (MAGIC+127)
      ts2: (t1 - MAGIC) max -127  -> int8
    The fp32 add of MAGIC = 1.5*2^23 forces round-to-nearest-even onto the
    integer grid (matching jnp.round); all other steps are exact in fp32.
    (The bias cannot be folded into MAGIC: at 2^23 the fp32 ulp is 1.0, so
    bias + MAGIC would round the bias to an integer before it meets the
    data.)
  - Host divides by 127 and transposes back to NHWC.
"""
import sys

import numpy as np

try:
    import concourse.bacc as bacc_mod
except ImportError:
    sys.path.insert(0, "/opt/trn_rl_repo")
    import concourse.bacc as bacc_mod

import concourse.mybir as mybir
from concourse.tile import TileContext
from concourse.bass_utils import run_bass_kernel_spmd
from contextlib import ExitStack

POOL = 8
NB = 8            # images per pool group (64 / 8)
H = W = 56
HO = WO = 54
C = 256
NCH = 2           # channel chunks of 128
NPIX = H * W      # 3136
NOUT = HO * WO    # 2916
NBANK = 6         # psum bank tiles per plane (6 * 486 = 2916)
BANKN = 486       # output pixels per psum tile (9 rows x 54)
MAGIC = 12582912.0  # 1.5 * 2^23
S = 127.0

f32 = mybir.dt.float32
f16 = mybir.dt.float16
i8 = mybir.dt.int8
Alu = mybir.AluOpType
Act = mybir.ActivationFunctionType

_cached = {}


def _build():
    nc = bacc_mod.Bacc()
    xt = nc.dram_tensor("xt", [NB, NCH, 128, NPIX], f16, kind="ExternalInput")
    wdg = nc.dram_tensor("wdg", [NCH, 9, 128, 128], f16, kind="ExternalInput")
    bv = nc.dram_tensor("bv", [NCH, 128, 1], f32, kind="ExternalInput")
    out = nc.dram_tensor("out", [NB, NCH, 128, NOUT], i8, kind="ExternalOutput")

    with TileContext(nc) as tc, ExitStack() as ctx:
        consts = ctx.enter_context(tc.tile_pool(name="consts", bufs=1))
        xpool = ctx.enter_context(tc.tile_pool(name="xpool", bufs=3))
        tpool = ctx.enter_context(tc.tile_pool(name="tpool", bufs=2))
        opool = ctx.enter_context(tc.tile_pool(name="opool", bufs=2))
        pspool = ctx.enter_context(tc.tile_pool(name="pspool", bufs=2, space="PSUM"))

        ws = consts.tile([128, NCH, 9, 128], f16)
        nc.sync.dma_start(out=ws, in_=wdg.rearrange("q t k m -> k q t m"))
        bt = consts.tile([128, NCH, 1], f32)
        nc.sync.dma_start(out=bt, in_=bv.rearrange("q k o -> k q o"))

        for n in range(NB):
            for q in range(NCH):
                xs = xpool.tile([128, NPIX], f16, tag="xs")
                nc.sync.dma_start(out=xs, in_=xt[n, q])
                xr = xs.rearrange("p (h w) -> p h w", w=W)
                y = tpool.tile([128, NOUT], f32, tag="y")
                for half in range(2):
                    banks = range(3 * half, 3 * half + 3)
                    pss = [pspool.tile([128, BANKN], f32, tag=f"ps{i}",
                                       name=f"ps{i}")
                           for i in range(3)]
                    for t in range(9):
                        i, j = divmod(t, 3)
                        for bi, b in enumerate(banks):
                            rhs = xr[:, 9 * b + i: 9 * b + i + 9, j: j + 54]
                            nc.tensor.matmul(pss[bi], lhsT=ws[:, q, t, :],
                                             rhs=rhs, start=(t == 0),
                                             stop=(t == 8),
                                             skip_group_check=True)
                    for bi, b in enumerate(banks):
                        nc.scalar.activation(
                            out=y[:, BANKN * b: BANKN * (b + 1)],
                            in_=pss[bi], func=Act.Identity,
                            bias=bt[:, q], scale=1.0)
                t1 = tpool.tile([128, NOUT], f32, tag="t1")
                nc.vector.tensor_scalar(out=t1, in0=y, scalar1=MAGIC,
                                        scalar2=MAGIC + S, op0=Alu.add,
                                        op1=Alu.min)
                ot = opool.tile([128, NOUT], i8, tag="ot")
                nc.vector.tensor_scalar(out=ot, in0=t1, scalar1=MAGIC,
                                        scalar2=-S, op0=Alu.subtract,
                                        op1=Alu.max)
                nc.sync.dma_start(out=out[n, q], in_=ot)

    nc.finalize()
    return nc


def kernel(X, W, bias, Werr, Berr, _trace=False):
    X = np.asarray(X, np.float32)
    W = np.asarray(W, np.float32)
    bias = np.asarray(bias, np.float32)
    Werr = np.asarray(Werr, np.float32)
    Berr = np.asarray(Berr, np.float32)

    if "nc" not in _cached:
        _cached["nc"] = _build()
    nc = _cached["nc"]

    Xh = X.astype(np.float16)  # [64, 56, 56, 256]
    w3 = W[..., 0]             # [3, 3, 256]
    we3 = Werr[..., 0]         # [8, 3, 3, 256]

    in_maps = []
    for p in range(POOL):
        xp = Xh[p * NB:(p + 1) * NB].reshape(NB, NPIX, C)
        xp = np.ascontiguousarray(xp.transpose(0, 2, 1)).reshape(NB, NCH, 128, NPIX)

        w_eff = (np.float32(S) * w3 * we3[p]).astype(np.float16)  # [3, 3, 256]
        wdg = np.zeros((NCH, 9, 128, 128), np.float16)
        for q in range(NCH):
            for t in range(9):
                i, j = divmod(t, 3)
                np.fill_diagonal(wdg[q, t], w_eff[i, j, 128 * q:128 * (q + 1)])

        b_eff = (np.float32(S) * bias * Berr[p]).astype(np.float32)
        in_maps.append({"xt": xp, "wdg": wdg, "bv": b_eff.reshape(NCH, 128, 1)})

    res = run_bass_kernel_spmd(nc, in_maps, core_ids=list(range(POOL)),
                               trace=_trace)
    if _trace:
        _cached["last_result"] = res

    outs = []
    for p in range(POOL):
        o = res.results[p]["out"].astype(np.float32)  # [NB, NCH, 128, NOUT] int8
        o = o / np.float32(S)
        o = o.reshape(NB, C, HO, WO).transpose(0, 2, 3, 1)  # NHWC
        outs.append(o)
    return np.ascontiguousarray(np.concatenate(outs, axis=0).astype(np.float32))
